# revision 1
# baseline (speedup 1.0000x reference)
"""Multi-head attention forward on 8 Trainium2 NeuronCores.

Problem (hardcoded): B=4, N=M=2048, D=1024, H=16, HS=64, OUT=1024, fp32.

Sharding: 8 cores = 4 batches x 2 head-groups of 8 heads. Each core
computes a partial output [2048, 1024] = sum over its 8 heads of
softmax((X_q Wq_h)(X_k Wk_h)^T / 8) (X_v Wv_h) Wo_h.  Host sums the two
head-group partials per batch and adds the projection bias.

Per-core kernel structure:
  1. Streamed load+transpose+project per input (q, k, v), software
     pipelined: projections of 512-row chunk c interleave with the PE
     transposes of chunk c+1, so the PE never runs a transpose-only
     stretch long enough (>3.4us) for HAM to re-throttle the clock.
  2. QT/KT [128(=2 heads x 64), 2048] per head-pair; V [128(m), 8, 65]
     f32r with a ones column at 64 (softmax denominator trick).
  3. Attention: one continuous stream over (pair, head, n-half, m-tile)
     with ctx trailing logits by PIPE steps and no breaks at block
     boundaries. Per-block 12-matmul warm bursts keep/restore the PE
     clock at 2.4 GHz. exp on ScalarE (scale=1/8, no max subtraction:
     |logits/8| < ~6).
  4. Block tails stage the denominator row (DVE copy + partition-hop
     DMA) and evict ctx rows un-normalized. Normalization per pair is
     deferred and split: shifts+reciprocal a couple of steps after the
     pair completes, PE-broadcast+multiply a few steps later, so PSUM
     ring borrows never wait on the reciprocal.
  5. Output projection: lhsT = ctxn pair n-block (K=128), rhs = Wo pair.
"""

import os
import sys

import numpy as np

for _p in ("/opt/trn_rl_repo",):
    if _p not in sys.path and os.path.isdir(_p):
        sys.path.insert(0, _p)

B, N, M, D = 4, 2048, 2048, 1024
H, HS, OUT = 16, 64, 1024
HL = 8          # heads per core
P = 128
NPAIR = HL // 2  # head pairs per core
DT = D // P      # 8 d-tiles
NT = N // P      # 16 n-tiles
MT = M // P      # 16 m-tiles
NH = 1024        # n-half width processed per attention block
PIPE = 3         # ctx trails logits by PIPE m-tiles


def build_mha(tc, ins, out_ap):
    import concourse.bass as bass
    from concourse import mybir

    nc = tc.nc
    f32 = mybir.dt.float32
    f32r = mybir.dt.float32r
    f16 = mybir.dt.float16

    xq, xk, xv = ins["xq"], ins["xk"], ins["xv"]
    wq, wk, wv, wo = ins["wq"], ins["wk"], ins["wv"], ins["wo"]

    import contextlib

    with contextlib.ExitStack() as ctx:
        # ---- constant tiles ----
        const = ctx.enter_context(tc.tile_pool(name="const", bufs=1))
        identity = const.tile([P, P], f32)
        from concourse.masks import make_identity
        make_identity(nc, identity)
        identity_r = const.tile([P, P], f32r)
        nc.vector.tensor_copy(identity_r[:], identity[:])
        ones_f32 = const.tile([P, HL], f32)
        nc.vector.memset(ones_f32[:], 1.0)
        ones_h = const.tile([P, HL], f16)
        nc.vector.tensor_copy(ones_h[:], ones_f32[:])
        identity_h = const.tile([P, P], f16)
        nc.vector.tensor_copy(identity_h[:], identity[:])
        # head-select mask: hmask2[0:2, s, :] is 1 on partition s, else 0.
        # K=2 lhsT for broadcasting one denominator row of a pair's sums tile
        # to 64 partitions.
        hmask2 = const.tile([2, 2, 64], f32)
        nc.gpsimd.memset(hmask2[:], 0.0)
        nc.gpsimd.affine_select(
            out=hmask2[:],
            in_=hmask2[:],
            compare_op=mybir.AluOpType.not_equal,
            fill=1.0,
            base=0,
            # iota = partition - s : zero exactly where partition == s
            pattern=[[-1, 2], [0, 64]],
            channel_multiplier=1,
        )
        hmask2_h = const.tile([2, 2, 64], f16)
        nc.vector.tensor_copy(hmask2_h[:], hmask2[:])

        # ---- persistent activations ----
        act_pool = ctx.enter_context(tc.tile_pool(name="acts", bufs=1))
        # QT/KT: one [128, 2048] tile per head pair; partitions 0:64 head 2p,
        # 64:128 head 2p+1.
        qt = [act_pool.tile([P, N], f32r, name=f"qt{p}", tag=f"qt{p}") for p in range(NPAIR)]
        kt = [act_pool.tile([P, M], f32r, name=f"kt{p}", tag=f"kt{p}") for p in range(NPAIR)]
        # V: per m-tile [128, 8 heads, 66] fp16; col 64 is ones (denominator).
        v_all = [act_pool.tile([P, HL, 66], f16, name=f"v{t}", tag=f"v{t}") for t in range(MT)]

        # wo [8, 64, 1024] -> SBUF [128(s*64+o), pair, 1024] (persistent; used
        # in phase 4).
        wo_sb = act_pool.tile([P, NPAIR, OUT], f32r, name="wo_sb", tag="wo_sb")

        # ---- phase 1+2: streamed load, transpose, project ----
        with tc.tile_pool(name="xt", bufs=2) as xt_pool, \
             tc.tile_pool(name="x_stream", bufs=4) as x_stream, \
             tc.tile_pool(name="wstage", bufs=4) as wstage_pool, \
             tc.tile_pool(name="wostage", bufs=1) as wostage_pool, \
             tc.tile_pool(name="tp_psum", bufs=4, space="PSUM") as tp_psum, \
             tc.tile_pool(name="proj_psum", bufs=4, space="PSUM") as proj_psum:

            def load_wo():
                # emitted at the END of phase 1+2: wo is not needed until
                # the output projection, and its DMAs otherwise gate the
                # first x-tile rounding copies on the scalar queue
                wo_stage = wostage_pool.tile(
                    [P, NPAIR, OUT], f32, name="wo_stage", tag="wost")
                for s in range(2):
                    nc.scalar.dma_start(
                        wo_stage[s * 64:(s + 1) * 64, :, :],
                        wo[s::2, :, :].rearrange("pp o d -> o pp d"))
                nc.vector.tensor_copy(wo_sb[:], wo_stage[:])

            def load_w(w_dram, pool, nm):
                # w [8, 1024, 64] -> SBUF [128(d in tile), dt, h, 64] (f32r)
                w_sb = pool.tile([P, DT, HL, HS], f32r, name=nm, tag=nm)
                for dt_i in range(DT):
                    w_stage = wstage_pool.tile([P, HL, HS], f32, name="w_stage", tag="wst")
                    nc.scalar.dma_start(
                        w_stage[:],
                        w_dram[:, dt_i * P:(dt_i + 1) * P, :].rearrange("h p o -> p h o"))
                    nc.vector.tensor_copy(w_sb[:, dt_i, :, :], w_stage[:])
                return w_sb

            def stream_input(x_dram, w_sb, kind):
                # software pipeline: transposes of chunk c+1 interleave with
                # the projection matmuls of chunk c.
                NC = NT // 4
                xt_tiles = {}

                def do_transpose(c, j):
                    if j == 0:
                        xt_tiles[c] = xt_pool.tile(
                            [P, DT, 512], f32r, name="xt_c", tag="xt_c")
                    xt_c = xt_tiles[c]
                    x_t = x_stream.tile([P, D], f32, name="x_t", tag="x_t")
                    nc.sync.dma_start(
                        x_t[:], x_dram[(4 * c + j) * P:(4 * c + j + 1) * P, :])
                    # round to f32r on the (idle in this phase) ScalarE so
                    # both the transpose LDWEIGHTS and matmul run single-pass
                    x_tr = x_stream.tile([P, D], f32r, name="x_tr", tag="x_tr")
                    nc.scalar.copy(x_tr[:], x_t[:])
                    for dt_i in range(DT):
                        tp = tp_psum.tile([P, P], f32r, name="tp", tag="tp")
                        nc.tensor.transpose(
                            tp[:], x_tr[:, dt_i * P:(dt_i + 1) * P], identity_r[:])
                        nc.vector.tensor_copy(
                            xt_c[:, dt_i, j * P:(j + 1) * P], tp[:])

                def do_proj(c, j):
                    xt_c = xt_tiles[c]
                    if kind == "v":
                        # V proj per m-tile: [128 m, 8 heads * 64] -> v_all
                        t = 4 * c + j
                        ps = proj_psum.tile([P, 512], f32, name="pp", tag="pp")
                        for dt_i in range(DT):
                            nc.tensor.matmul(
                                ps[:],
                                xt_c[:, dt_i, j * P:(j + 1) * P],
                                w_sb[:, dt_i, :, :],
                                start=(dt_i == 0), stop=(dt_i == DT - 1),
                            )
                        nc.vector.tensor_copy(
                            v_all[t][:, :, 0:64],
                            ps[:].rearrange("p (h o) -> p h o", h=HL))
                        nc.vector.tensor_copy(
                            v_all[t][:, :, 64:65],
                            ones_h[:, 0:HL].rearrange("p (h one) -> p h one", one=1))
                    else:
                        dst = qt if kind == "q" else kt
                        p = j
                        ps = proj_psum.tile([P, 512], f32, name="pp", tag="pp")
                        for dt_i in range(DT):
                            nc.tensor.matmul(
                                ps[:],
                                w_sb[:, dt_i, 2 * p:2 * p + 2, :],
                                xt_c[:, dt_i, :],
                                start=(dt_i == 0), stop=(dt_i == DT - 1),
                            )
                        nc.vector.tensor_copy(
                            dst[p][:, c * 512:(c + 1) * 512], ps[:])

                for j in range(4):
                    do_transpose(0, j)
                for c in range(NC):
                    for j in range(4):
                        if c + 1 < NC:
                            do_transpose(c + 1, j)
                        do_proj(c, j)
                    del xt_tiles[c]

            with tc.tile_pool(name="wq_pool", bufs=1) as wq_pool:
                stream_input(xq, load_w(wq, wq_pool, "wq_sb"), "q")
            with tc.tile_pool(name="wk_pool", bufs=1) as wk_pool:
                stream_input(xk, load_w(wk, wk_pool, "wk_sb"), "k")
            with tc.tile_pool(name="wv_pool", bufs=1) as wv_pool:
                stream_input(xv, load_w(wv, wv_pool, "wv_sb"), "v")
            load_wo()

        # ---- phase 3: attention, one continuous stream ----
        ctxn_pool = ctx.enter_context(tc.tile_pool(name="ctxn_pool", bufs=1))
        ctxn = [ctxn_pool.tile([P, N], f32r, name=f"ctxn{p}", tag=f"ctxn{p}")
                for p in range(NPAIR)]

        with tc.tile_pool(name="lgs", bufs=3) as lgs_pool, \
             tc.tile_pool(name="sstage", bufs=2) as sstage_pool, \
             tc.tile_pool(name="lg_psum", bufs=3, space="PSUM") as lg_psum, \
             tc.tile_pool(name="ctx_psum", bufs=1, space="PSUM") as ctx_psum, \
             tc.tile_pool(name="misc", bufs=2) as misc_pool:

            # per-pair denominator rows: [2, N] tiles (partition-0 based so
            # the per-pair reciprocal satisfies partition alignment). Each
            # pair's tile dies right after its normalization, so a 2-deep
            # ring suffices.
            sums_pair = {}

            def get_sums(p):
                if p not in sums_pair:
                    sums_pair[p] = misc_pool.tile(
                        [2, N], f32, name=f"sums{p}", tag="sums")
                return sums_pair[p]

            # blocks ordered so each pair finishes before the next starts:
            # pair-local order (s, nh) = (0,0),(0,1),(1,0),(1,1)
            blocks = [(p, s, nh)
                      for p in range(NPAIR) for s in range(2) for nh in range(2)]
            steps = [(bi, t) for bi in range(len(blocks)) for t in range(MT)]
            cps_map = {}
            ets = {}
            shift_tmp = {}

            def emit_warm(bi):
                # HAM warm-up/re-promotion burst: one gapless accumulation
                # chain (no inter-instruction waits) long enough to fill a
                # 4096-cycle activity window and promote the PE clock to
                # 2.4 GHz. Results are never read.
                p, s, nh = blocks[bi]
                warm = lg_psum.tile([P, NH], f32, name="warm", tag="lg")
                for w in range(10):
                    nc.tensor.matmul(
                        warm[:, 0:512],
                        kt[p][:, (w % MT) * P:((w % MT) + 1) * P],
                        qt[p][:, 0:512],
                        start=(w == 0), stop=(w == 9), skip_group_check=True,
                    )

            GW = 2  # t-steps per exp group

            def is_direct(bi, t):
                # all exps straight from PSUM: DVE staging raised total chip
                # power enough that the PE clock got clawed back harder than
                # the ScalarE overhead savings were worth
                return True

            def emit_logits(bi, t):
                p, s, nh = blocks[bi]
                prange = slice(s * 64, s * 64 + 64)
                n0 = nh * NH
                lg = lg_psum.tile([P, NH], f32, name="lg", tag="lg")
                for c in range(NH // 512):
                    nc.tensor.matmul(
                        lg[:, c * 512:(c + 1) * 512],
                        kt[p][prange, t * P:(t + 1) * P],
                        qt[p][prange, n0 + c * 512:n0 + (c + 1) * 512],
                        start=True, stop=True,
                    )
                if is_direct(bi, t):
                    et = lgs_pool.tile([P, NH], f16, name="etd", tag="etd")
                    nc.scalar.activation(
                        et[:], lg[:], mybir.ActivationFunctionType.Exp,
                        scale=0.125)
                    ets[(bi, t)] = et
                    return
                # stage PSUM->SBUF on DVE so exp can run 2048-wide (halves
                # ScalarE per-instruction overhead, which paces the stream)
                if t % GW == 0:
                    ets[(bi, t // GW)] = lgs_pool.tile(
                        [P, GW, NH], f16, name="lgs", tag="lgs")
                lgs = ets[(bi, t // GW)]
                nc.vector.tensor_copy(lgs[:, t % GW, :], lg[:])
                if t % GW == GW - 1:
                    nc.scalar.activation(
                        lgs[:], lgs[:],
                        mybir.ActivationFunctionType.Exp, scale=0.125)

            def emit_evict(bi):
                # stage the denominator row into the pair's sums tile (DVE
                # copy + partition-hop DMA); evict ctx rows UN-normalized.
                # Odd heads only stage into tmp here; the PE shift to
                # partitions 64:128 is deferred to emit_shift_recip.
                p, s, nh = blocks[bi]
                n0 = nh * NH
                cps = cps_map.pop(bi)
                sums_stage = sstage_pool.tile([P, NH], f32, name="sst", tag="sst")
                nc.vector.tensor_copy(sums_stage[64:65, :], cps[64:65, :])
                nc.sync.dma_start(
                    get_sums(p)[s:s + 1, n0:n0 + NH], sums_stage[64:65, :])
                if s == 0:
                    nc.vector.tensor_copy(ctxn[p][0:64, n0:n0 + NH], cps[0:64, :])
                else:
                    tmp = sstage_pool.tile([64, NH], f16, name="ctmp", tag="ctmp")
                    nc.vector.tensor_copy(tmp[:], cps[0:64, :])
                    shift_tmp[(p, nh)] = tmp

            def emit_shift_recip(p):
                # odd-head PE shifts (inputs long ready -> no ring wait) and
                # the pair's reciprocal (DVE, doesn't touch the PSUM rings)
                for nh in range(2):
                    n0 = nh * NH
                    tmp = shift_tmp.pop((p, nh))
                    sh = lg_psum.tile([P, NH], f32, name="lg", tag="lg")
                    for c in range(NH // 512):
                        nc.tensor.matmul(
                            sh[64:128, c * 512:(c + 1) * 512],
                            identity_h[0:64, 0:64],
                            tmp[:, c * 512:(c + 1) * 512],
                            start=True, stop=True,
                        )
                    nc.vector.tensor_copy(
                        ctxn[p][64:128, n0:n0 + NH], sh[64:128, :])
                sums = sums_pair[p]
                nc.vector.reciprocal_approx_fast(sums[:], sums[:])
                sums_h = sstage_pool.tile([2, N], f16, name="sums_h", tag="ctmp")
                nc.vector.tensor_copy(sums_h[:], sums[:])
                sums_h_map[p] = sums_h

            def emit_norm(p):
                # PE-broadcast the reciprocal rows (fp16, single-pass) to the
                # pair's 128 partitions and multiply into ctxn in place.
                # Emitted a few steps after emit_shift_recip so the lg-ring
                # borrow never waits on the reciprocal.
                sums_pair.pop(p)
                sums_h = sums_h_map.pop(p)
                for c2 in range(N // 1024):
                    c2sl = slice(c2 * 1024, (c2 + 1) * 1024)
                    bc = lg_psum.tile([P, 1024], f32, name="lg", tag="lg")
                    for s in range(2):
                        for c in range(2):
                            nc.tensor.matmul(
                                bc[s * 64:(s + 1) * 64, c * 512:(c + 1) * 512],
                                hmask2_h[:, s, :],
                                sums_h[:, c2 * 1024 + c * 512:c2 * 1024 + (c + 1) * 512],
                                start=True, stop=True,
                            )
                    nc.vector.tensor_mul(
                        ctxn[p][:, c2sl], ctxn[p][:, c2sl], bc[:])

            def emit_ctx(bi, t):
                p, s, nh = blocks[bi]
                hl = 2 * p + s
                if t == 0:
                    cps_map[bi] = ctx_psum.tile([P, NH], f32, name="cps", tag="cps")
                cps = cps_map[bi]
                if is_direct(bi, t):
                    et = ets.pop((bi, t))
                    etv = et[:, :]
                else:
                    et = ets[(bi, t // GW)]
                    etv = et[:, t % GW, :]
                for c in range(NH // 512):
                    nc.tensor.matmul(
                        cps[0:65, c * 512:(c + 1) * 512],
                        v_all[t][:, hl, 0:65],
                        etv[:, c * 512:(c + 1) * 512],
                        start=(t == 0), stop=(t == MT - 1),
                    )
                if not is_direct(bi, t) and t % GW == GW - 1:
                    ets.pop((bi, t // GW))
                if t == MT - 1:
                    emit_evict(bi)

            pending = []
            sums_h_map = {}
            for i, (bi, t) in enumerate(steps):
                if i == 0:
                    emit_warm(bi)
                emit_logits(bi, t)
                if i >= PIPE:
                    cbi, ct = steps[i - PIPE]
                    emit_ctx(cbi, ct)
                    if ct == MT - 1 and blocks[cbi][1] == 1 and blocks[cbi][2] == 1:
                        pr = blocks[cbi][0]
                        pending.append((i + 2, emit_shift_recip, pr))
                        pending.append((i + 6, emit_norm, pr))
                        pending.append((i + 8, lambda _pr, _bi=bi: emit_warm(min(_bi + 1, len(blocks) - 1)), pr))
                while pending and pending[0][0] <= i:
                    _, fn, pr = pending.pop(0)
                    fn(pr)
            for i in range(len(steps) - PIPE, len(steps)):
                emit_ctx(*steps[i])
            emit_shift_recip(NPAIR - 1)
            emit_norm(NPAIR - 1)

        # ---- phase 4: output projection ----
        with tc.tile_pool(name="out_psum", bufs=8, space="PSUM") as out_psum, \
             tc.tile_pool(name="out_sb", bufs=4) as out_pool:
            warm = out_psum.tile([P, 512], f32, name="ops", tag="ops")
            for w in range(10):
                nc.tensor.matmul(
                    warm[:],
                    ctxn[0][:, (w % MT) * P:((w % MT) + 1) * P],
                    wo_sb[:, 0, 0:512],
                    start=(w == 0), stop=(w == 9), skip_group_check=True,
                )
            for t in range(NT):
                ot = out_pool.tile([P, OUT], f32, name="ot", tag="ot")
                for c in range(OUT // 512):
                    ops = out_psum.tile([P, 512], f32, name="ops", tag="ops")
                    for p in range(NPAIR):
                        nc.tensor.matmul(
                            ops[:],
                            ctxn[p][:, t * P:(t + 1) * P],
                            wo_sb[:, p, c * 512:(c + 1) * 512],
                            start=(p == 0), stop=(p == NPAIR - 1),
                        )
                    # alternate eviction engines so neither gates slot reuse
                    if (2 * t + c) % 2 == 0:
                        nc.scalar.copy(ot[:, c * 512:(c + 1) * 512], ops[:])
                    else:
                        nc.vector.tensor_copy(ot[:, c * 512:(c + 1) * 512], ops[:])
                nc.sync.dma_start(out_ap[t * P:(t + 1) * P, :], ot[:])


def build_nc():
    import concourse.bacc as bacc
    import concourse.tile as tile
    from concourse import mybir

    nc = bacc.Bacc("TRN2", target_bir_lowering=False, debug=False)
    f32 = mybir.dt.float32
    ins = {
        "xq": nc.dram_tensor("xq", (N, D), f32, kind="ExternalInput").ap(),
        "xk": nc.dram_tensor("xk", (M, D), f32, kind="ExternalInput").ap(),
        "xv": nc.dram_tensor("xv", (M, D), f32, kind="ExternalInput").ap(),
        "wq": nc.dram_tensor("wq", (HL, D, HS), f32, kind="ExternalInput").ap(),
        "wk": nc.dram_tensor("wk", (HL, D, HS), f32, kind="ExternalInput").ap(),
        "wv": nc.dram_tensor("wv", (HL, D, HS), f32, kind="ExternalInput").ap(),
        "wo": nc.dram_tensor("wo", (HL, HS, OUT), f32, kind="ExternalInput").ap(),
    }
    out_ap = nc.dram_tensor("out", (N, OUT), f32, kind="ExternalOutput").ap()
    with tile.TileContext(nc) as tc:
        build_mha(tc, ins, out_ap)
    nc.compile()
    return nc


def make_in_maps(inputs):
    q = np.ascontiguousarray(np.asarray(inputs["query"], dtype=np.float32))
    k = np.ascontiguousarray(np.asarray(inputs["key"], dtype=np.float32))
    v = np.ascontiguousarray(np.asarray(inputs["value"], dtype=np.float32))
    wq = np.asarray(inputs["query_kernel"], dtype=np.float32)
    wk = np.asarray(inputs["key_kernel"], dtype=np.float32)
    wv = np.asarray(inputs["value_kernel"], dtype=np.float32)
    wo = np.asarray(inputs["projection_kernel"], dtype=np.float32)
    in_maps = []
    for c in range(8):
        b, hg = divmod(c, 2)
        hs = slice(hg * HL, (hg + 1) * HL)
        in_maps.append({
            "xq": q[b], "xk": k[b], "xv": v[b],
            "wq": np.ascontiguousarray(wq[hs]),
            "wk": np.ascontiguousarray(wk[hs]),
            "wv": np.ascontiguousarray(wv[hs]),
            "wo": np.ascontiguousarray(wo[hs]),
        })
    return in_maps


def combine(results, bias):
    out = np.empty((B, N, OUT), dtype=np.float32)
    for b in range(B):
        out[b] = results[2 * b]["out"] + results[2 * b + 1]["out"]
    out += np.asarray(bias, dtype=np.float32)[None, None, :]
    return out


_NC_CACHE = None
_LDW_PATCHED = False


def _enable_ldw_opt():
    """walrus dedupes back-to-back LDWEIGHTS of the same stationary only
    with --enable-ldw-opt=true; concourse pins it false. Our inner loops
    issue pairs of matmuls sharing one stationary, so the reload costs
    ~300ns each on the TensorE critical path."""
    # No-op now: the fp16 65-column ctx weights are incompatible with the
    # walrus LDW dedupe pass, and stationary reuse is minimal in the current
    # structure (single wide-moving ctx matmuls), so the opt no longer pays.
    return


def kernel(**inputs):
    global _NC_CACHE
    from concourse import bass_utils
    _enable_ldw_opt()

    if _NC_CACHE is None:
        _NC_CACHE = build_nc()
    nc = _NC_CACHE
    in_maps = make_in_maps(inputs)
    res = bass_utils.run_bass_kernel_spmd(nc, in_maps, core_ids=list(range(8)))
    return combine(res.results, inputs["projection_bias"])



# revision 5
# speedup vs baseline: 1.2436x; 1.2436x over previous
"""Multi-head attention forward on 8 Trainium2 NeuronCores.

Problem (hardcoded): B=4, N=M=2048, D=1024, H=16, HS=64, OUT=1024, fp32.

Sharding: 8 cores = 4 batches x 2 head-groups of 8 heads. Each core
computes a partial output [2048, 1024] = sum over its 8 heads of
softmax((X_q Wq_h)(X_k Wk_h)^T / 8) (X_v Wv_h) Wo_h.  Host sums the two
head-group partials per batch and adds the projection bias.

v2 design (vs the f32r baseline):
  * All-fp16 PE data path (x, weights, qt/kt, V, exp, ctxn): single-pass
    matmuls with FWL-eligible 128-col stationaries, fp16 transposes.
  * Logits for the two heads of a pair run CONCURRENTLY as row-tiled
    K=64 matmuls (tile_position (0,0) / (64,0)) -- the m-tile stationary
    kt[0:64]/kt[64:128] pair streams both moving q halves in the same
    512-cycle window, halving logits PE time.
  * exp once per step over both heads' logits [128, 2x512] on ScalarE
    (the ACT engine is the attention pacer at ~1 elem/lane/cycle).
  * ctx odd-head partition placement via SBUF->SBUF partition-hop DMA
    (idle DMA engines) instead of PE shift matmuls.
  * Per-(pair, n-quarter) deferred normalization so the final pair's
    norm tail is short; out-projection n-quarters emit as soon as all
    pairs' ctxn quarters are normalized.
  * No HAM warm bursts: the instruction stream has no PE gap long
    enough to re-throttle the clock.
"""

import os
import sys

import numpy as np

for _p in ("/opt/trn_rl_repo",):
    if _p not in sys.path and os.path.isdir(_p):
        sys.path.insert(0, _p)

B, N, M, D = 4, 2048, 2048, 1024
H, HS, OUT = 16, 64, 1024
HL = 8          # heads per core
P = 128
NPAIR = HL // 2  # head pairs per core
DT = D // P      # 8 d-tiles
NT = N // P      # 16 n-tiles
MT = M // P      # 16 m-tiles
NQ = 512         # n-quarter width per attention block
PIPE = 2         # ctx trails logits by PIPE m-tiles


def build_mha(tc, ins, out_ap):
    import concourse.bass as bass
    from concourse import mybir

    nc = tc.nc
    f32 = mybir.dt.float32
    f16 = mybir.dt.float16

    xq, xk, xv = ins["xq"], ins["xk"], ins["xv"]
    wq, wk, wv, wo = ins["wq"], ins["wk"], ins["wv"], ins["wo"]

    import contextlib

    with contextlib.ExitStack() as ctx:
        # ---- constant tiles ----
        const = ctx.enter_context(tc.tile_pool(name="const", bufs=1))
        identity = const.tile([P, P], f32)
        from concourse.masks import make_identity
        make_identity(nc, identity)
        identity_h = const.tile([P, P], f16)
        nc.vector.tensor_copy(identity_h[:], identity[:])
        ones_f32 = const.tile([P, HL], f32)
        nc.vector.memset(ones_f32[:], 1.0)
        ones_h = const.tile([P, HL], f16)
        nc.vector.tensor_copy(ones_h[:], ones_f32[:])
        # head-select mask: hmask2[0:2, s, :] is 1 on partition s, else 0.
        # K=2 lhsT for broadcasting one denominator row of a pair's sums tile
        # to 64 partitions.
        hmask2 = const.tile([2, 2, 64], f32)
        nc.gpsimd.memset(hmask2[:], 0.0)
        nc.gpsimd.affine_select(
            out=hmask2[:],
            in_=hmask2[:],
            compare_op=mybir.AluOpType.not_equal,
            fill=1.0,
            base=0,
            # iota = partition - s : zero exactly where partition == s
            pattern=[[-1, 2], [0, 64]],
            channel_multiplier=1,
        )
        hmask2_h = const.tile([2, 2, 64], f16)
        nc.vector.tensor_copy(hmask2_h[:], hmask2[:])

        # ---- persistent activations ----
        act_pool = ctx.enter_context(tc.tile_pool(name="acts", bufs=1))
        # QT/KT: one [128, 2048] fp16 tile per head pair; partitions 0:64
        # head 2p, 64:128 head 2p+1.
        qt = [act_pool.tile([P, N], f16, name=f"qt{p}", tag=f"qt{p}") for p in range(NPAIR)]
        kt = [act_pool.tile([P, M], f16, name=f"kt{p}", tag=f"kt{p}") for p in range(NPAIR)]
        # V: per m-tile [128, 8 heads, 66] fp16; col 64 is ones (denominator).
        v_all = [act_pool.tile([P, HL, 66], f16, name=f"v{t}", tag=f"v{t}") for t in range(MT)]

        # wo [8, 64, 1024] -> SBUF [128(s*64+o), pair, 1024] fp16 (persistent;
        # used in the output projection).
        wo_sb = act_pool.tile([P, NPAIR, OUT], f16, name="wo_sb", tag="wo_sb")

        # ctx accumulators, un-normalized until per-quarter norm: fp16.
        ctxn = [act_pool.tile([P, N], f16, name=f"ctxn{p}", tag=f"ctxn{p}")
                for p in range(NPAIR)]

        # ---- phase 1+2: streamed load, transpose, project ----
        with tc.tile_pool(name="xt", bufs=2) as xt_pool, \
             tc.tile_pool(name="x_stream", bufs=4) as x_stream, \
             tc.tile_pool(name="wstage", bufs=4) as wstage_pool, \
             tc.tile_pool(name="wostage", bufs=1) as wostage_pool, \
             tc.tile_pool(name="tp_psum", bufs=4, space="PSUM") as tp_psum, \
             tc.tile_pool(name="proj_psum", bufs=4, space="PSUM") as proj_psum:

            def load_wo():
                # emitted at the END of phase 1+2: wo is not needed until
                # the output projection, and its DMAs otherwise gate the
                # first x-tile rounding copies on the scalar queue
                wo_stage = wostage_pool.tile(
                    [P, NPAIR, OUT], f32, name="wo_stage", tag="wost")
                for s in range(2):
                    nc.scalar.dma_start(
                        wo_stage[s * 64:(s + 1) * 64, :, :],
                        wo[s::2, :, :].rearrange("pp o d -> o pp d"))
                nc.vector.tensor_copy(wo_sb[:], wo_stage[:])

            def load_w(w_dram, pool, nm):
                # w [8, 1024, 64] -> SBUF [128(d in tile), dt, h, 64] (f16)
                w_sb = pool.tile([P, DT, HL, HS], f16, name=nm, tag=nm)
                for dt_i in range(DT):
                    w_stage = wstage_pool.tile([P, HL, HS], f32, name="w_stage", tag="wst")
                    nc.scalar.dma_start(
                        w_stage[:],
                        w_dram[:, dt_i * P:(dt_i + 1) * P, :].rearrange("h p o -> p h o"))
                    nc.vector.tensor_copy(w_sb[:, dt_i, :, :], w_stage[:])
                return w_sb

            def stream_input(x_dram, w_sb, kind):
                # software pipeline: transposes of chunk c+1 interleave with
                # the projection matmuls of chunk c.
                NC = NT // 4
                xt_tiles = {}

                def do_transpose(c, j):
                    if j == 0:
                        xt_tiles[c] = xt_pool.tile(
                            [P, DT, 512], f16, name="xt_c", tag="xt_c")
                    xt_c = xt_tiles[c]
                    x_t = x_stream.tile([P, D], f32, name="x_t", tag="x_t")
                    nc.sync.dma_start(
                        x_t[:], x_dram[(4 * c + j) * P:(4 * c + j + 1) * P, :])
                    # round to fp16 on the (otherwise idle in this phase)
                    # ScalarE so transpose + matmul run single-pass fp16
                    x_tr = x_stream.tile([P, D], f16, name="x_tr", tag="x_tr")
                    nc.scalar.copy(x_tr[:], x_t[:])
                    # 2 transpose groups of 4: each fills a [128, 512] fp16
                    # psum tile, evicted by one wide DVE copy
                    for g in range(2):
                        tp = tp_psum.tile([P, 4, P], f16, name="tp", tag="tp")
                        for q in range(4):
                            dt_i = 4 * g + q
                            nc.tensor.transpose(
                                tp[:, q, :], x_tr[:, dt_i * P:(dt_i + 1) * P],
                                identity_h[:])
                        nc.vector.tensor_copy(
                            xt_c[:, 4 * g:4 * g + 4, j * P:(j + 1) * P],
                            tp[:])

                def do_proj(c, j):
                    xt_c = xt_tiles[c]
                    if kind == "v":
                        # V proj per m-tile: [128 m, 8 heads * 64] -> v_all
                        t = 4 * c + j
                        ps = proj_psum.tile([P, 512], f32, name="pp", tag="pp")
                        for dt_i in range(DT):
                            nc.tensor.matmul(
                                ps[:],
                                xt_c[:, dt_i, j * P:(j + 1) * P],
                                w_sb[:, dt_i, :, :],
                                start=(dt_i == 0), stop=(dt_i == DT - 1),
                            )
                        nc.vector.tensor_copy(
                            v_all[t][:, :, 0:64],
                            ps[:].rearrange("p (h o) -> p h o", h=HL))
                        nc.vector.tensor_copy(
                            v_all[t][:, :, 64:65],
                            ones_h[:, 0:HL].rearrange("p (h one) -> p h one", one=1))
                    else:
                        dst = qt if kind == "q" else kt
                        p = j
                        ps = proj_psum.tile([P, 512], f32, name="pp", tag="pp")
                        for dt_i in range(DT):
                            nc.tensor.matmul(
                                ps[:],
                                w_sb[:, dt_i, 2 * p:2 * p + 2, :],
                                xt_c[:, dt_i, :],
                                start=(dt_i == 0), stop=(dt_i == DT - 1),
                            )
                        nc.vector.tensor_copy(
                            dst[p][:, c * 512:(c + 1) * 512], ps[:])

                for j in range(4):
                    do_transpose(0, j)
                for c in range(NC):
                    for j in range(4):
                        if c + 1 < NC:
                            do_transpose(c + 1, j)
                        do_proj(c, j)
                    del xt_tiles[c]

            with tc.tile_pool(name="wk_pool", bufs=1) as wk_pool:
                stream_input(xk, load_w(wk, wk_pool, "wk_sb"), "k")
            with tc.tile_pool(name="wv_pool", bufs=1) as wv_pool:
                stream_input(xv, load_w(wv, wv_pool, "wv_sb"), "v")
            with tc.tile_pool(name="wq_pool", bufs=1) as wq_pool:
                stream_input(xq, load_w(wq, wq_pool, "wq_sb"), "q")
            load_wo()

        # ---- phase 3: attention, one continuous stream ----
        # blocks = (pair, n-quarter); steps = m-tiles. Per step both heads
        # of the pair are processed: row-tiled concurrent logits, one wide
        # exp, two ctx accumulation matmuls.
        # PSUM budget (8 banks): lg ring 2 x 2 banks + cps0/cps1 1 bank each
        # + norm broadcast 1 + out-proj 1 = 8.
        with tc.tile_pool(name="lgs", bufs=4) as lgs_pool, \
             tc.tile_pool(name="sstage", bufs=3) as sstage_pool, \
             tc.tile_pool(name="sums", bufs=2) as sums_pool, \
             tc.tile_pool(name="lg_psum", bufs=2, space="PSUM") as lg_psum, \
             tc.tile_pool(name="ctx_psum", bufs=1, space="PSUM") as ctx_psum, \
             tc.tile_pool(name="nrm_psum", bufs=1, space="PSUM") as nrm_psum, \
             tc.tile_pool(name="out_psum", bufs=1, space="PSUM") as out_psum, \
             tc.tile_pool(name="out_sb", bufs=4) as out_pool:

            # per-pair denominator rows: [2, N] f32 tiles (partition 0 = even
            # head, 1 = odd head), filled by partition-hop DMAs per quarter.
            sums_pair = {}
            sums_h_pair = {}

            def get_sums(p):
                if p not in sums_pair:
                    sums_pair[p] = sums_pool.tile(
                        [2, N], f32, name=f"sums{p}", tag="sums")
                    sums_h_pair[p] = sums_pool.tile(
                        [2, N], f16, name=f"sumsh{p}", tag="sumsh")
                return sums_pair[p]

            blocks = [(p, nq) for p in range(NPAIR) for nq in range(N // NQ)]
            steps = [(bi, t) for bi in range(len(blocks)) for t in range(MT)]
            cps_map = {}
            ets = {}

            def emit_logits(bi, t):
                p, nq = blocks[bi]
                n0 = nq * NQ
                lg = lg_psum.tile([P, 2, NQ], f32, name="lg", tag="lg")
                # both heads of the pair, concurrently (row tiling: head 0 in
                # PE rows 0:64, head 1 in rows 64:128, separate PSUM banks)
                nc.tensor.matmul(
                    lg[:, 0, :],
                    kt[p][0:64, t * P:(t + 1) * P],
                    qt[p][0:64, n0:n0 + NQ],
                    start=True, stop=True,
                )
                nc.tensor.matmul(
                    lg[:, 1, :],
                    kt[p][64:128, t * P:(t + 1) * P],
                    qt[p][64:128, n0:n0 + NQ],
                    start=True, stop=True,
                )
                et = lgs_pool.tile([P, 2, NQ], f16, name="et", tag="et")
                nc.scalar.activation(
                    et[:, :, :], lg[:, :, :],
                    mybir.ActivationFunctionType.Exp, scale=0.125)
                ets[(bi, t)] = et

            def emit_evict(bi):
                # stage denominator rows into the pair's sums tile (DVE copy
                # + partition-hop DMA); evict ctx rows UN-normalized. Head 1
                # hops partitions 0:64 -> 64:128 via SBUF->SBUF DMA.
                p, nq = blocks[bi]
                n0 = nq * NQ
                cps0, cps1 = cps_map.pop(bi)
                sums = get_sums(p)
                sstage = sstage_pool.tile([P, 2, NQ], f32, name="sst", tag="sst")
                nc.vector.tensor_copy(sstage[64:65, 0, :], cps0[64:65, :])
                nc.vector.tensor_copy(sstage[64:65, 1, :], cps1[64:65, :])
                nc.sync.dma_start(sums[0:1, n0:n0 + NQ], sstage[64:65, 0, :])
                nc.sync.dma_start(sums[1:2, n0:n0 + NQ], sstage[64:65, 1, :])
                nc.vector.tensor_copy(ctxn[p][0:64, n0:n0 + NQ], cps0[0:64, :])
                tmp = sstage_pool.tile([64, NQ], f16, name="ctmp", tag="ctmp")
                nc.vector.tensor_copy(tmp[:], cps1[0:64, :])
                nc.sync.dma_start(ctxn[p][64:128, n0:n0 + NQ], tmp[:])

            def emit_norm(bi):
                # per-(pair, quarter): reciprocal of the staged denominators,
                # PE-broadcast (fp16, K=2) to the pair's 128 partitions, then
                # multiply into ctxn in place.
                p, nq = blocks[bi]
                n0 = nq * NQ
                sums = sums_pair[p]
                sums_h = sums_h_pair[p]
                nc.vector.reciprocal_approx_fast(
                    sums[:, n0:n0 + NQ], sums[:, n0:n0 + NQ])
                nc.vector.tensor_copy(sums_h[:, n0:n0 + NQ], sums[:, n0:n0 + NQ])
                bc = nrm_psum.tile([P, NQ], f32, name="bc", tag="bc")
                for s in range(2):
                    nc.tensor.matmul(
                        bc[s * 64:(s + 1) * 64, :],
                        hmask2_h[:, s, :],
                        sums_h[:, n0:n0 + NQ],
                        start=True, stop=True,
                    )
                nc.vector.tensor_mul(
                    ctxn[p][:, n0:n0 + NQ], ctxn[p][:, n0:n0 + NQ], bc[:])

            def emit_ctx(bi, t):
                p, nq = blocks[bi]
                if t == 0:
                    cps_map[bi] = (
                        ctx_psum.tile([P, NQ], f32, name="cps0", tag="cps0"),
                        ctx_psum.tile([P, NQ], f32, name="cps1", tag="cps1"),
                    )
                cps0, cps1 = cps_map[bi]
                et = ets.pop((bi, t))
                nc.tensor.matmul(
                    cps0[0:65, :],
                    v_all[t][:, 2 * p, 0:65],
                    et[:, 0, :],
                    start=(t == 0), stop=(t == MT - 1),
                )
                nc.tensor.matmul(
                    cps1[0:65, :],
                    v_all[t][:, 2 * p + 1, 0:65],
                    et[:, 1, :],
                    start=(t == 0), stop=(t == MT - 1),
                )
                if t == MT - 1:
                    emit_evict(bi)

            def emit_outproj_quarter(nq):
                # out projection for the 4 n-tiles of quarter nq; all pairs'
                # ctxn quarters are normalized by the time this is emitted.
                for tj in range(4):
                    tile_n = nq * 4 + tj
                    ot = out_pool.tile([P, OUT], f32, name="ot", tag="ot")
                    for c in range(OUT // 512):
                        ops = out_psum.tile([P, 512], f32, name="ops", tag="ops")
                        for p in range(NPAIR):
                            nc.tensor.matmul(
                                ops[:],
                                ctxn[p][:, tile_n * P:(tile_n + 1) * P],
                                wo_sb[:, p, c * 512:(c + 1) * 512],
                                start=(p == 0), stop=(p == NPAIR - 1),
                            )
                        # alternate eviction engines so neither gates reuse
                        if (2 * tile_n + c) % 2 == 0:
                            nc.scalar.copy(ot[:, c * 512:(c + 1) * 512], ops[:])
                        else:
                            nc.vector.tensor_copy(ot[:, c * 512:(c + 1) * 512], ops[:])
                    nc.sync.dma_start(out_ap[tile_n * P:(tile_n + 1) * P, :], ot[:])

            # quarter-norm readiness bookkeeping: norm for block bi fires a
            # couple of steps after its evict; once the LAST pair's quarter
            # nq is normalized, out-proj for quarter nq can be emitted.
            pending = []
            normed = set()
            outproj_done = set()

            def run_pending_entry(what, arg):
                if what == "norm":
                    emit_norm(arg)
                    normed.add(arg)
                else:
                    emit_outproj_quarter(arg)
                    outproj_done.add(arg)

            for i, (bi, t) in enumerate(steps):
                emit_logits(bi, t)
                if i >= PIPE:
                    cbi, ct = steps[i - PIPE]
                    emit_ctx(cbi, ct)
                    if ct == MT - 1:
                        # evict just ran for cbi; norm a couple steps later
                        # (after the partition-hop DMAs land)
                        pending.append((i + 4, "norm", cbi))
                        if blocks[cbi][0] == NPAIR - 1:
                            pending.append((i + 6, "outproj", blocks[cbi][1]))
                while pending and pending[0][0] <= i:
                    _, what, arg = pending.pop(0)
                    run_pending_entry(what, arg)
            for i in range(len(steps) - PIPE, len(steps)):
                emit_ctx(*steps[i])
            for _, what, arg in pending:
                run_pending_entry(what, arg)
            for bi in range(len(blocks)):
                if bi not in normed:
                    emit_norm(bi)
            for nq in range(N // NQ):
                if nq not in outproj_done:
                    emit_outproj_quarter(nq)


def build_nc():
    import concourse.bacc as bacc
    import concourse.tile as tile
    from concourse import mybir

    nc = bacc.Bacc("TRN2", target_bir_lowering=False, debug=False)
    f32 = mybir.dt.float32
    ins = {
        "xq": nc.dram_tensor("xq", (N, D), f32, kind="ExternalInput").ap(),
        "xk": nc.dram_tensor("xk", (M, D), f32, kind="ExternalInput").ap(),
        "xv": nc.dram_tensor("xv", (M, D), f32, kind="ExternalInput").ap(),
        "wq": nc.dram_tensor("wq", (HL, D, HS), f32, kind="ExternalInput").ap(),
        "wk": nc.dram_tensor("wk", (HL, D, HS), f32, kind="ExternalInput").ap(),
        "wv": nc.dram_tensor("wv", (HL, D, HS), f32, kind="ExternalInput").ap(),
        "wo": nc.dram_tensor("wo", (HL, HS, OUT), f32, kind="ExternalInput").ap(),
    }
    out_ap = nc.dram_tensor("out", (N, OUT), f32, kind="ExternalOutput").ap()
    with tile.TileContext(nc) as tc:
        build_mha(tc, ins, out_ap)
    nc.compile()
    return nc


def make_in_maps(inputs):
    q = np.ascontiguousarray(np.asarray(inputs["query"], dtype=np.float32))
    k = np.ascontiguousarray(np.asarray(inputs["key"], dtype=np.float32))
    v = np.ascontiguousarray(np.asarray(inputs["value"], dtype=np.float32))
    wq = np.asarray(inputs["query_kernel"], dtype=np.float32)
    wk = np.asarray(inputs["key_kernel"], dtype=np.float32)
    wv = np.asarray(inputs["value_kernel"], dtype=np.float32)
    wo = np.asarray(inputs["projection_kernel"], dtype=np.float32)
    in_maps = []
    for c in range(8):
        b, hg = divmod(c, 2)
        hs = slice(hg * HL, (hg + 1) * HL)
        in_maps.append({
            "xq": q[b], "xk": k[b], "xv": v[b],
            "wq": np.ascontiguousarray(wq[hs]),
            "wk": np.ascontiguousarray(wk[hs]),
            "wv": np.ascontiguousarray(wv[hs]),
            "wo": np.ascontiguousarray(wo[hs]),
        })
    return in_maps


def combine(results, bias):
    out = np.empty((B, N, OUT), dtype=np.float32)
    for b in range(B):
        out[b] = results[2 * b]["out"] + results[2 * b + 1]["out"]
    out += np.asarray(bias, dtype=np.float32)[None, None, :]
    return out


_NC_CACHE = None


def _enable_ldw_opt():
    # kept as a no-op hook for test.py compatibility
    return


def kernel(**inputs):
    global _NC_CACHE
    from concourse import bass_utils
    _enable_ldw_opt()

    if _NC_CACHE is None:
        _NC_CACHE = build_nc()
    nc = _NC_CACHE
    in_maps = make_in_maps(inputs)
    res = bass_utils.run_bass_kernel_spmd(nc, in_maps, core_ids=list(range(8)))
    return combine(res.results, inputs["projection_bias"])
